# revision 1
# baseline (speedup 1.0000x reference)
"""Self-contained TRN2 Bass kernel for the nn_Attention problem.

kernel(**inputs) takes the FULL inputs (x [8,1024,1024], W_qkv, b_qkv, W_out,
b_out), shards batch-parallel across 8 NeuronCores (one batch element per
core), runs a causal multi-head-attention kernel per core, and returns the
full [8, 1024, 1024] float32 output.

Per-core pipeline (all matmuls in fp32r at full PE rate, fp32 accumulation):
  A: xT = transpose(x) via PE-transpose tiles
  B: qkT = W_qk^T @ xT; v = xT^T @ W_v (+ a ones column per head for the
     softmax denominator)
  C: per head-pair, causal scoresT chunks -> ACT exp -> gpsimd triangle mask
     -> [V|1]^T @ exp accumulation; denominators ride in psum row 64;
     normalized with DVE reciprocal + DMA partition-broadcast
  D: y = attn_outT^T @ W_out + b_out
Softmax skips the max-subtraction (scores/8 are bounded ~3 for this problem),
which allows reducing along the PSUM partition axis with a ones-column matmul.
"""

import os
import sys

for _p in ("/opt/trn_rl_repo", os.path.expanduser("~/.axon_site/_ro/trn_rl_repo")):
    if os.path.isdir(_p) and _p not in sys.path:
        sys.path.insert(0, _p)

from contextlib import ExitStack

import numpy as np

import concourse.bass as bass
import concourse.tile as tile
from concourse import bacc, mybir
from concourse.masks import make_identity

F32 = mybir.dt.float32
F32R = mybir.dt.float32r

S = 1024
D = 1024
H = 16
DH = 64
P = 128
NQ = 512  # q-chunk (matmul moving free dim)
SC = S // P  # 8 sequence chunks of 128
DC = D // P  # 8 model-dim chunks of 128
MQK = 2 * D // P  # 16 row-chunks of qkT


def build_kernel(use_f32r=True, niter=1, psS_bufs=2, psO_bufs=4, exp_bufs=8, wq_bufs=3, psb_bufs=6, stage_psO=False, phases='abcd', uniform=True, unpaired=False, c_dma_gp=False, noact=False, pair_nq=False, split_exp=False, early_psS=False, pair2=False):
    nc = bacc.Bacc("TRN2", target_bir_lowering=False, debug=False, num_devices=8)

    x_ap = nc.dram_tensor("x", [S, D], F32, kind="ExternalInput").ap()
    wqkv_ap = nc.dram_tensor("W_qkv", [D, 3 * D], F32, kind="ExternalInput").ap()
    bqkv_ap = nc.dram_tensor("b_qkv", [3 * D], F32, kind="ExternalInput").ap()
    wout_ap = nc.dram_tensor("W_out", [D, D], F32, kind="ExternalInput").ap()
    bout_ap = nc.dram_tensor("b_out", [D], F32, kind="ExternalInput").ap()
    y_ap = nc.dram_tensor("y", [S, D], F32, kind="ExternalOutput").ap()

    DT = F32R if use_f32r else F32

    def r(ap):
        return ap

    with tile.TileContext(nc) as tc:
      for _it in range(niter):
        top = ExitStack()
        p_top = top.enter_context(tc.tile_pool(name="p_top", bufs=1))

        ident = p_top.tile([P, P], F32)
        make_identity(nc, ident)

        # per-partition bias view of b_qkv rows (rows of qkvT): [p, m]
        bqkv_sb = p_top.tile([P, 3 * D // P], F32)
        nc.sync.dma_start(bqkv_sb[:], bqkv_ap.rearrange("(m p) -> p m", p=P))
        qkT = p_top.tile([P, MQK, S], DT)  # [p, m, s]

        # ---------------- Phase A: xT = transpose(x) -------------------
        with tc.tile_pool(name="p_ab", bufs=1) as p_ab:
            xT = p_ab.tile([P, DC, S], DT)  # [p, dd, s] = x[s, 128*dd+p]
            wqkv_r = wqkv_ap.rearrange("(kc p) n -> p kc n", p=P)
            with tc.tile_pool(name="wv", bufs=1) as wvp:
                wv = wvp.tile([P, DC, D], DT)  # W_qkv[128kc+p, 2048+n]

                with tc.tile_pool(name="xload", bufs=3) as xpool, tc.tile_pool(
                    name="pst", bufs=4, space="PSUM"
                ) as pst:
                    for so in range(SC):
                        x_t = xpool.tile([P, D], F32, tag="x")
                        nc.sync.dma_start(x_t[:], x_ap[so * P : (so + 1) * P, :])
                        for dd in range(DC):
                            ps = pst.tile([P, P], F32, tag="pt")
                            nc.tensor.transpose(
                                ps[:], x_t[:, dd * P : (dd + 1) * P], ident[:]
                            )
                            if dd % 2 == 0:
                                nc.scalar.copy(
                                    xT[:, dd, so * P : (so + 1) * P], ps[:]
                                )
                            else:
                                nc.vector.tensor_copy(
                                    xT[:, dd, so * P : (so + 1) * P], ps[:]
                                )

                # ------------- Phase B1: qkT = W_qk^T @ xT --------------
                with tc.tile_pool(name="wq", bufs=wq_bufs) as wqp, tc.tile_pool(
                    name="psb", bufs=6, space="PSUM"
                ) as psb:
                    for m in range(MQK):
                        wq = wqp.tile([P, DC, P], DT, tag="wq")
                        nc.sync.dma_start(wq[:], wqkv_r[:, :, m * P : (m + 1) * P].bitcast(DT))
                        if m % 2 == 0 and m // 2 < DC:
                            kc = m // 2  # spread the W_v prefetch across B1
                            nc.sync.dma_start(
                                wv[:, kc, :], wqkv_r[:, kc, 2 * D :].bitcast(DT)
                            )
                        if pair_nq:
                            pss = [
                                psb.tile([P, NQ], F32, tag="ps", name=f"psb_{m}_{nq}")
                                for nq in range(S // NQ)
                            ]
                        for kc in range(DC):
                            if not pair_nq:
                                break
                            for nq in range(S // NQ):
                                nc.tensor.matmul(
                                    pss[nq][:],
                                    r(wq[:, kc, :]),
                                    r(xT[:, kc, nq * NQ : (nq + 1) * NQ]),
                                    start=(kc == 0),
                                    stop=(kc == DC - 1),
                                )
                        for nq in range(S // NQ):
                            if pair_nq:
                                ps = pss[nq]
                            else:
                                ps = psb.tile([P, NQ], F32, tag="ps")
                                for kc in range(DC):
                                    nc.tensor.matmul(
                                        ps[:],
                                        r(wq[:, kc, :]),
                                        r(xT[:, kc, nq * NQ : (nq + 1) * NQ]),
                                        start=(kc == 0),
                                        stop=(kc == DC - 1),
                                    )
                            if m % 2 == 0:
                                nc.vector.tensor_scalar(
                                    out=qkT[:, m, nq * NQ : (nq + 1) * NQ],
                                    in0=ps[:],
                                    scalar1=bqkv_sb[:, m : m + 1],
                                    scalar2=None,
                                    op0=mybir.AluOpType.add,
                                )
                            else:
                                nc.scalar.add(
                                    qkT[:, m, nq * NQ : (nq + 1) * NQ],
                                    ps[:],
                                    bqkv_sb[:, m : m + 1],
                                )

                # ------------- Phase B2: v = xT^T @ W_v (+ones cols) ----
                v_sb = p_top.tile([P, SC, H * 65], DT)  # [p, so, 65h+c]
                # ones columns (65th of each head's block)
                ones_view = v_sb[:].rearrange("p so (h c) -> p so h c", c=65)[
                    :, :, :, 64
                ]
                nc.vector.tensor_copy(
                    ones_view, nc.const_aps.tensor(1.0, list(ones_view.shape), F32)
                )
                psS_early = None
                if early_psS:
                    psS_early = tc.alloc_tile_pool(
                        name="psSe", bufs=psS_bufs, space="PSUM", side="right"
                    )
                p_b2 = tc.alloc_tile_pool(name="p_b2", bufs=1)
                biasv_bc = p_b2.tile([P, D], F32)
                nc.sync.dma_start(
                    biasv_bc[:], bqkv_ap[2 * D :][None, :].to_broadcast((P, D))
                )
                psb2 = tc.alloc_tile_pool(
                    name="psb2", bufs=(4 if early_psS else psb_bufs), space="PSUM"
                )
                for so in range(SC):
                    if pair_nq:
                        ps2s = [
                            psb2.tile([P, NQ], F32, tag="ps2", name=f"ps2_{so}_{nq}")
                            for nq in range(D // NQ)
                        ]
                        for kc in range(DC):
                            for nq in range(D // NQ):
                                nc.tensor.matmul(
                                    ps2s[nq][:],
                                    r(xT[:, kc, so * P : (so + 1) * P]),
                                    r(wv[:, kc, nq * NQ : (nq + 1) * NQ]),
                                    start=(kc == 0),
                                    stop=(kc == DC - 1),
                                )
                    for nq in range(D // NQ):
                        if pair_nq:
                            ps = ps2s[nq]
                        else:
                            ps = psb2.tile([P, NQ], F32, tag="ps2")
                            for kc in range(DC):
                                nc.tensor.matmul(
                                    ps[:],
                                    r(xT[:, kc, so * P : (so + 1) * P]),
                                    r(wv[:, kc, nq * NQ : (nq + 1) * NQ]),
                                    start=(kc == 0),
                                    stop=(kc == DC - 1),
                                )
                        # strided dest: per head 64 V columns (ones col untouched)
                        dest = v_sb[:, so, :].rearrange("p (h c) -> p h c", c=65)[
                            :, 8 * nq : 8 * nq + 8, 0:64
                        ]
                        nc.vector.tensor_tensor(
                            out=dest,
                            in0=ps[:].rearrange("p (h c) -> p h c", c=64),
                            in1=biasv_bc[:, nq * NQ : (nq + 1) * NQ].rearrange(
                                "p (h c) -> p h c", c=64
                            ),
                            op=mybir.AluOpType.add,
                        )

                psb2.release()
                p_b2.release()

        if phases == "ab":
            for qc in range(SC):
                nc.sync.dma_start(
                    y_ap[qc * P : (qc + 1) * P, :], qkT[:, qc, :].bitcast(F32)
                )
            top.close()
            continue

        # ---------------- Phase C: attention ---------------------------
        attnT = p_top.tile([P, DC, S], DT)  # [p, dd, s] rows of attn_out^T
        wop = tc.alloc_tile_pool(name="wo", bufs=1)
        with ExitStack() as cs:
            # prefetch W_out (phase D) in two column halves while C runs
            wout_r = wout_ap.rearrange("(kc p) n -> p kc n", p=P)
            wo_half = []
            for half in range(2):
                woh = wop.tile([P, DC, NQ], DT, name=f"wo_{half}", tag=f"wo{half}")
                nc.sync.dma_start(
                    woh[:], wout_r[:, :, half * NQ : (half + 1) * NQ].bitcast(DT)
                )
                wo_half.append(woh)

            epool = cs.enter_context(tc.tile_pool(name="exp", bufs=exp_bufs))
            if early_psS:
                psS = psS_early
            else:
                psS = cs.enter_context(
                    tc.tile_pool(name="psS", bufs=psS_bufs, space="PSUM")
                )
            psO = cs.enter_context(tc.tile_pool(name="psO", bufs=psO_bufs, space="PSUM"))
            rpool = cs.enter_context(tc.tile_pool(name="rp", bufs=2))
            drpool = cs.enter_context(tc.tile_pool(name="dr", bufs=2, space="DRAM"))
            bcpool = cs.enter_context(tc.tile_pool(name="bc", bufs=2))
            stpool = cs.enter_context(tc.tile_pool(name="st", bufs=2))
            e0 = None
            if noact:
                e0 = rpool.tile([P, 2 * NQ], DT, tag="e0s", name="e0_static")
                nc.vector.tensor_copy(
                    e0[:], nc.const_aps.tensor(1.0, [P, 2 * NQ], F32)
                )

            pair_orders = {}
            for j in range(S // NQ):
                if pair2:
                    # emit pairs in AB-interleaved order: 0,1,0,1 chunks...
                    pass
                nkc = (j + 1) * NQ // P  # k chunks needed (causal)
                for pair in range(H // 2):
                    m = pair
                    halves = [(0, 2 * pair), (64, 2 * pair + 1)]  # (base, head)
                    po = {}
                    for base, h in halves:
                        po[h] = psO.tile([65, NQ], F32, tag="psO", name=f"psO_{j}_{h}")
                    for i in range(nkc):
                        i_loc = i - 4 * j
                        dead = max(0, i_loc * P)  # causally-dead columns per half
                        if uniform:
                            dead = 0
                        if unpaired:
                            # per-head 1-bank psum + contiguous 2D exp/affine
                            es = {}
                            for idx, (base, h) in enumerate(halves):
                                psh = psS.tile(
                                    [P, NQ], F32, tag="psS",
                                    name=f"psSu_{j}_{m}_{i}_{idx}",
                                )
                                nc.tensor.matmul(
                                    psh[:],
                                    r(qkT[base : base + 64, 8 + m, i * P : (i + 1) * P]),
                                    r(qkT[base : base + 64, m, j * NQ : (j + 1) * NQ]),
                                    start=True,
                                    stop=True,
                                )
                                eh = epool.tile(
                                    [P, NQ], DT, tag="exp",
                                    name=f"eu_{j}_{m}_{i}_{idx}",
                                )
                                nc.scalar.activation(
                                    eh[:],
                                    psh[:],
                                    mybir.ActivationFunctionType.Exp,
                                    scale=0.125,
                                )
                                if i_loc >= 0:
                                    band = i_loc * P
                                    nc.gpsimd.affine_select(
                                        out=eh[:, 0 : band + P],
                                        in_=eh[:, 0 : band + P],
                                        compare_op=mybir.AluOpType.is_ge,
                                        fill=0.0,
                                        base=-band,
                                        pattern=[[1, band + P]],
                                        channel_multiplier=-1,
                                    )
                                es[h] = eh
                            for idx, (base, h) in enumerate(halves):
                                nc.tensor.matmul(
                                    po[h][:],
                                    r(v_sb[:, i, 65 * h : 65 * h + 65]),
                                    r(es[h][:]),
                                    start=(i == 0),
                                    stop=(i == nkc - 1),
                                )
                            continue
                        # both heads' scoresT into one 2-bank psum tile
                        ps = psS.tile([P, 2 * NQ], F32, tag="psS", name=f"psS_{j}_{m}_{i}")
                        for idx, (base, h) in enumerate(halves):
                            nc.tensor.matmul(
                                ps[:, idx * NQ + dead : (idx + 1) * NQ],
                                r(qkT[base : base + 64, 8 + m, i * P : (i + 1) * P]),
                                r(qkT[base : base + 64, m, j * NQ + dead : (j + 1) * NQ]),
                                start=True,
                                stop=True,
                            )
                        if noact:
                            for idx, (base, h) in enumerate(halves):
                                nc.tensor.matmul(
                                    po[h][:],
                                    r(v_sb[:, i, 65 * h : 65 * h + 65]),
                                    r(e0[:, idx * NQ : (idx + 1) * NQ]),
                                    start=(i == 0),
                                    stop=(i == nkc - 1),
                                )
                            continue
                        e = epool.tile([P, 2 * NQ], DT, tag="exp", name=f"e_{j}_{m}_{i}")
                        ps_v = ps[:].rearrange("p (g c) -> p g c", c=NQ)
                        e_v = e[:].rearrange("p (g c) -> p g c", c=NQ)
                        if split_exp:
                            for g in range(2):
                                nc.scalar.activation(
                                    e_v[:, g, dead:],
                                    ps_v[:, g, dead:],
                                    mybir.ActivationFunctionType.Exp,
                                    scale=0.125,
                                )
                        else:
                            nc.scalar.activation(
                                e_v[:, :, dead:],
                                ps_v[:, :, dead:],
                                mybir.ActivationFunctionType.Exp,
                                scale=0.125,
                            )
                        if i_loc >= 0:
                            # mask the diagonal band in place: keep e[x, y]
                            # only where (y - band_off) - x >= 0
                            band = max(0, i_loc * P)
                            nc.gpsimd.affine_select(
                                out=e_v[:, :, dead : band + P],
                                in_=e_v[:, :, dead : band + P],
                                compare_op=mybir.AluOpType.is_ge,
                                fill=0.0,
                                base=-(band - dead),
                                pattern=[[0, 2], [1, band + P - dead]],
                                channel_multiplier=-1,
                            )
                        for idx, (base, h) in enumerate(halves):
                            # columns < dead get no contribution from this k
                            # chunk (causally dead) -- skip them in the matmul
                            nc.tensor.matmul(
                                po[h][:, dead:],
                                r(v_sb[:, i, 65 * h : 65 * h + 65]),
                                r(e[:, idx * NQ + dead : (idx + 1) * NQ]),
                                start=(i == 0),
                                stop=(i == nkc - 1),
                            )
                    for base, h in halves:
                        if stage_psO:
                            # drain psO to SBUF immediately to free the bank
                            og = rpool.tile([65, NQ], F32, tag="og", name=f"og_{j}_{h}")
                            nc.vector.tensor_copy(og[:], po[h][:])
                            src_o = og
                        else:
                            src_o = po[h]
                        dma_eng = nc.gpsimd if c_dma_gp else nc.sync
                        rt = rpool.tile([65, NQ], F32, tag="r", name=f"r_{j}_{h}")
                        nc.vector.reciprocal(rt[64:65, :], src_o[64:65, :])
                        scr = drpool.tile([NQ], F32, tag="scr", name=f"scr_{j}_{h}")
                        dma_eng.dma_start(scr[None, :], rt[64:65, :])
                        rbc = bcpool.tile([64, NQ], F32, tag="rbc", name=f"rbc_{j}_{h}")
                        dma_eng.dma_start(rbc[:], scr[None, :].to_broadcast((64, NQ)))
                        if base == 0:
                            nc.vector.tensor_tensor(
                                out=attnT[0:64, m, j * NQ : (j + 1) * NQ],
                                in0=src_o[0:64, :],
                                in1=rbc[:],
                                op=mybir.AluOpType.mult,
                            )
                        else:
                            st = stpool.tile([64, NQ], DT, tag="st", name=f"st_{j}_{h}")
                            nc.vector.tensor_tensor(
                                out=st[:],
                                in0=src_o[0:64, :],
                                in1=rbc[:],
                                op=mybir.AluOpType.mult,
                            )
                            dma_eng.dma_start(
                                attnT[64:128, m, j * NQ : (j + 1) * NQ], st[:]
                            )

        # ------------- Phase D: y = attnT^T @ W_out + b_out --------
        with tc.tile_pool(name="psY", bufs=4, space="PSUM") as psY, tc.tile_pool(
            name="yp", bufs=3
        ) as ypool:
            boutbc = ypool.tile([P, D], F32)
            nc.sync.dma_start(boutbc[:], bout_ap[None, :].to_broadcast((P, D)))
            for qc in range(SC):
                if pair_nq:
                    psys = [
                        psY.tile([P, NQ], F32, tag="psY", name=f"psY_{nq}_{qc}")
                        for nq in range(D // NQ)
                    ]
                    for kc in range(DC):
                        for nq in range(D // NQ):
                            nc.tensor.matmul(
                                psys[nq][:],
                                r(attnT[:, kc, qc * P : (qc + 1) * P]),
                                r(wo_half[nq][:, kc, :]),
                                start=(kc == 0),
                                stop=(kc == DC - 1),
                            )
                for nq in range(D // NQ):
                    if pair_nq:
                        ps = psys[nq]
                    else:
                        ps = psY.tile([P, NQ], F32, tag="psY", name=f"psY_{nq}_{qc}")
                        for kc in range(DC):
                            nc.tensor.matmul(
                                ps[:],
                                r(attnT[:, kc, qc * P : (qc + 1) * P]),
                                r(wo_half[nq][:, kc, :]),
                                start=(kc == 0),
                                stop=(kc == DC - 1),
                            )
                    yt = ypool.tile([P, NQ], F32, tag="y", name=f"y_{nq}_{qc}")
                    nc.vector.tensor_tensor(
                        out=yt[:],
                        in0=ps[:],
                        in1=boutbc[:, nq * NQ : (nq + 1) * NQ],
                        op=mybir.AluOpType.add,
                    )
                    nc.sync.dma_start(
                        y_ap[qc * P : (qc + 1) * P, nq * NQ : (nq + 1) * NQ], yt[:]
                    )

        wop.release()
        if early_psS:
            psS_early.release()
        top.close()

    nc.compile()
    return nc




_CACHED = {}


def _get_nc():
    if "nc" not in _CACHED:
        _CACHED["nc"] = build_kernel(use_f32r=True, niter=1)
    return _CACHED["nc"]


def kernel(x, W_qkv, b_qkv, W_out, b_out):
    x = np.ascontiguousarray(np.asarray(x, dtype=np.float32))
    W_qkv = np.ascontiguousarray(np.asarray(W_qkv, dtype=np.float32))
    b_qkv = np.ascontiguousarray(np.asarray(b_qkv, dtype=np.float32))
    W_out = np.ascontiguousarray(np.asarray(W_out, dtype=np.float32))
    b_out = np.ascontiguousarray(np.asarray(b_out, dtype=np.float32))
    B = x.shape[0]
    assert x.shape == (8, S, D), f"expected x [8, {S}, {D}], got {x.shape}"

    from concourse.bass_utils import run_bass_kernel_spmd

    nc = _get_nc()
    in_maps = [
        {
            "x": np.ascontiguousarray(x[b]),
            "W_qkv": W_qkv,
            "b_qkv": b_qkv,
            "W_out": W_out,
            "b_out": b_out,
        }
        for b in range(B)
    ]
    res = run_bass_kernel_spmd(nc, in_maps, list(range(B)))
    return np.stack([res.results[b]["y"] for b in range(B)]).astype(np.float32)



# revision 27
# speedup vs baseline: 1.0301x; 1.0301x over previous
"""Self-contained TRN2 Bass kernel for the nn_Attention problem.

kernel(**inputs) takes the FULL inputs (x [8,1024,1024], W_qkv, b_qkv, W_out,
b_out), shards batch-parallel across 8 NeuronCores (one batch element per
core), runs a causal multi-head-attention kernel per core, and returns the
full [8, 1024, 1024] float32 output.

Per-core pipeline:
  A: xT = transpose(x) via PE-transpose tiles (f32r)
  B1: qkT = W_qk^T @ xT (bf16 out), nq-outer so it starts after half of x
      is transposed; all 16 W_qk column tiles held in SBUF. DMAs are emitted
      in need order: x[0:4], all W_qk, x[4:8], W_v, W_out.
  B2: v = xT^T @ W_v (bf16, + a ones column per head for the denominator)
  C: per head-pair, causal scoresT chunks (bf16 matmuls, causally-dead
     columns skipped) -> ACT exp (bf16 out) -> DVE triangle-mask multiply on
     the 128-wide diagonal band -> [V|1]^T @ exp accumulation; denominators
     ride in psum row 64; normalized with DVE reciprocal + gpsimd
     partition-broadcast (no DMA). After each query window of C, the
     corresponding phase-D output chunks are emitted to fill PE bubbles.
  D: y = attnT^T @ W_out + b_out (f32r), interleaved into C per window.
Softmax skips the max-subtraction (scores/8 are bounded ~3 for this problem),
which allows reducing along the PSUM partition axis with a ones-column matmul.
"""

import os
import sys

for _p in ("/opt/trn_rl_repo", os.path.expanduser("~/.axon_site/_ro/trn_rl_repo")):
    if os.path.isdir(_p) and _p not in sys.path:
        sys.path.insert(0, _p)

from contextlib import ExitStack

import numpy as np

import concourse.bass as bass
import concourse.tile as tile
from concourse import bacc, mybir
from concourse.masks import make_identity

F32 = mybir.dt.float32
F32R = mybir.dt.float32r
BF16 = mybir.dt.bfloat16

S = 1024
D = 1024
H = 16
DH = 64
P = 128
NQ = 512  # q-chunk (matmul moving free dim)
SC = S // P  # 8 sequence chunks of 128
DC = D // P  # 8 model-dim chunks of 128
MQK = 2 * D // P  # 16 row-chunks of qkT


def build_kernel(use_f32r=True, niter=1, psS_bufs=2, psO_bufs=3, psY_bufs=1,
                 exp_bufs=8, interleave_d=True, **_ignored):
    nc = bacc.Bacc("TRN2", target_bir_lowering=False, debug=False, num_devices=8)

    x_ap = nc.dram_tensor("x", [S, D], F32, kind="ExternalInput").ap()
    wqkv_ap = nc.dram_tensor("W_qkv", [D, 3 * D], F32, kind="ExternalInput").ap()
    bqkv_ap = nc.dram_tensor("b_qkv", [3 * D], F32, kind="ExternalInput").ap()
    wout_ap = nc.dram_tensor("W_out", [D, D], F32, kind="ExternalInput").ap()
    bout_ap = nc.dram_tensor("b_out", [D], F32, kind="ExternalInput").ap()
    y_ap = nc.dram_tensor("y", [S, D], F32, kind="ExternalOutput").ap()

    DT = F32R if use_f32r else F32
    wqkv_r = wqkv_ap.rearrange("(kc p) n -> p kc n", p=P)
    wout_r = wout_ap.rearrange("(kc p) n -> p kc n", p=P)

    with tile.TileContext(nc) as tc:
      for _it in range(niter):
        top = ExitStack()
        p_top = top.enter_context(tc.tile_pool(name="p_top", bufs=1))

        ident_f = p_top.tile([P, P], F32)
        make_identity(nc, ident_f)
        ident = p_top.tile([P, P], DT)  # f32r copy for the f32r transposes
        nc.vector.tensor_copy(ident[:], ident_f[:])

        # duplicated causal triangle tile for the two halves of an e tile:
        # tri2[p, g*P + c] = 1 if c >= p else 0
        tri2 = p_top.tile([P, 2 * P], BF16)
        nc.vector.tensor_copy(tri2[:], nc.const_aps.tensor(1.0, [P, 2 * P], F32))
        nc.gpsimd.affine_select(
            out=tri2[:],
            in_=tri2[:],
            compare_op=mybir.AluOpType.is_ge,
            fill=0.0,
            base=0,
            pattern=[[0, 2], [1, P]],
            channel_multiplier=-1,
        )

        # per-partition bias view of b_qkv rows (rows of qkvT): [p, m]
        bqkv_sb = p_top.tile([P, 3 * D // P], F32)
        nc.sync.dma_start(bqkv_sb[:], bqkv_ap.rearrange("(m p) -> p m", p=P))
        qkT = p_top.tile([P, MQK, S], BF16)  # [p, m, s]
        v_sb = p_top.tile([P, SC, H * 65], BF16)  # [p, so, 65h+c]

        # ---------------- Phases A+B ----------------------------------
        p_ab = tc.alloc_tile_pool(name="p_ab", bufs=1)
        xT = p_ab.tile([P, DC, S], DT)  # [p, dd, s] = x[s, 128*dd+p]
        wvp = tc.alloc_tile_pool(name="wv", bufs=1)
        wv = wvp.tile([P, DC, D], DT)  # W_qkv[128kc+p, 2048+n]

        # --- DMA emission in need order ---
        xpool = tc.alloc_tile_pool(name="xload", bufs=4)
        x_tiles = {}
        for so in range(4):
            x_t = xpool.tile([P, D], DT, tag="x", name=f"x_{so}")
            nc.sync.dma_start(x_t[:], x_ap[so * P : (so + 1) * P, :].bitcast(DT))
            x_tiles[so] = x_t
        wqp = tc.alloc_tile_pool(name="wq", bufs=1)
        wq_all = []
        for m in range(MQK):
            wq = wqp.tile([P, DC, P], DT, tag=f"wq{m}", name=f"wq_{m}")
            nc.sync.dma_start(wq[:], wqkv_r[:, :, m * P : (m + 1) * P].bitcast(DT))
            wq_all.append(wq)
        for so in range(4, SC):
            x_t = xpool.tile([P, D], DT, tag="x", name=f"x_{so}")
            nc.sync.dma_start(x_t[:], x_ap[so * P : (so + 1) * P, :].bitcast(DT))
            x_tiles[so] = x_t
        for kc in range(DC):
            nc.sync.dma_start(wv[:, kc, :], wqkv_r[:, kc, 2 * D :].bitcast(DT))

        # --- Phases A+B1 interleaved: transpose the x chunks a query
        # window needs, then run B1 for that window while the next window's
        # x chunks stream in ---------------------------------------------
        with tc.tile_pool(name="pst", bufs=4, space="PSUM") as pst, tc.tile_pool(
            name="psb", bufs=4, space="PSUM"
        ) as psb:
            for nq in range(S // NQ):
                for so in range(4 * nq, 4 * nq + 4):
                    x_t = x_tiles[so]
                    for dg in range(DC // 4):  # 4 transposes per psum bank
                        ps = pst.tile([P, 4 * P], DT, tag="pt")
                        for dl in range(4):
                            dd = dg * 4 + dl
                            nc.tensor.transpose(
                                ps[:, dl * P : (dl + 1) * P],
                                x_t[:, dd * P : (dd + 1) * P],
                                ident[:],
                            )
                        dest = xT[:, dg * 4 : dg * 4 + 4, so * P : (so + 1) * P]
                        src = ps[:].rearrange("p (dl c) -> p dl c", c=P)
                        if dg % 2 == 0:
                            nc.scalar.copy(dest, src)
                        else:
                            nc.vector.tensor_copy(dest, src)
                for m in range(MQK):
                    ps = psb.tile([P, NQ], F32, tag="ps")
                    for kc in range(DC):
                        nc.tensor.matmul(
                            ps[:],
                            wq_all[m][:, kc, :],
                            xT[:, kc, nq * NQ : (nq + 1) * NQ],
                            start=(kc == 0),
                            stop=(kc == DC - 1),
                        )
                    if m % 2 == 0:
                        nc.vector.tensor_scalar(
                            out=qkT[:, m, nq * NQ : (nq + 1) * NQ],
                            in0=ps[:],
                            scalar1=bqkv_sb[:, m : m + 1],
                            scalar2=None,
                            op0=mybir.AluOpType.add,
                        )
                    else:
                        nc.scalar.add(
                            qkT[:, m, nq * NQ : (nq + 1) * NQ],
                            ps[:],
                            bqkv_sb[:, m : m + 1],
                        )
        # release x tiles and wq tiles (keep xT, wv)
        wqp.release()
        xpool.release()

        # W_out prefetch (needed by D, interleaved into C) — emit after wv.
        # Loaded fp32, converted to bf16 during the idle pre-C window (attnT
        # and the D matmuls are bf16).
        wobp = tc.alloc_tile_pool(name="wob", bufs=1, side="right")
        wop = tc.alloc_tile_pool(name="wo", bufs=1, side="right")
        wo_f32 = []
        wo_half = []
        for half in range(2):
            woh = wop.tile([P, DC, NQ], F32, name=f"wo_{half}", tag=f"wo{half}")
            nc.sync.dma_start(woh[:], wout_r[:, :, half * NQ : (half + 1) * NQ])
            wo_f32.append(woh)
            wo_half.append(
                wobp.tile([P, DC, NQ], BF16, name=f"wob_{half}", tag=f"wob{half}")
            )

        # --- Phase B2: v = xT^T @ W_v (+ones cols) ---------------------
        # ones columns (65th of each head's block)
        ones_view = v_sb[:].rearrange("p so (h c) -> p so h c", c=65)[:, :, :, 64]
        nc.vector.tensor_copy(
            ones_view, nc.const_aps.tensor(1.0, list(ones_view.shape), F32)
        )
        biasv_bc = p_top.tile([P, D], F32)
        nc.sync.dma_start(
            biasv_bc[:], bqkv_ap[2 * D :][None, :].to_broadcast((P, D))
        )

        def emit_b2_so(so, pool, tag):
            for nq in range(D // NQ):
                ps = pool.tile([P, NQ], F32, tag=tag, name=f"psB2_{so}_{nq}")
                for kc in range(DC):
                    nc.tensor.matmul(
                        ps[:],
                        xT[:, kc, so * P : (so + 1) * P],
                        wv[:, kc, nq * NQ : (nq + 1) * NQ],
                        start=(kc == 0),
                        stop=(kc == DC - 1),
                    )
                # strided dest: per head 64 V columns (ones col untouched)
                dest = v_sb[:, so, :].rearrange("p (h c) -> p h c", c=65)[
                    :, 8 * nq : 8 * nq + 8, 0:64
                ]
                nc.vector.tensor_tensor(
                    out=dest,
                    in0=ps[:].rearrange("p (h c) -> p h c", c=64),
                    in1=biasv_bc[:, nq * NQ : (nq + 1) * NQ].rearrange(
                        "p (h c) -> p h c", c=64
                    ),
                    op=mybir.AluOpType.add,
                )

        # V for the first query window (keys 0-511) now; the rest is
        # interleaved into phase C's first window, which is ACT-bound
        psb2 = tc.alloc_tile_pool(name="psb2", bufs=6, space="PSUM")
        for so in range(4):
            emit_b2_so(so, psb2, "ps2")
        psb2.release()

        # W_out fp32 -> bf16 while ACT/DVE are idle between B2 and C
        nc.scalar.copy(wo_half[0][:], wo_f32[0][:])
        nc.vector.tensor_copy(wo_half[1][:], wo_f32[1][:])
        wop.release()

        # ---------------- Phase C (+ interleaved D) --------------------
        with ExitStack() as cs:
            p_c = cs.enter_context(tc.tile_pool(name="p_c", bufs=1))
            attnT = p_c.tile([P, DC, S], BF16)  # [p, dd, s] rows of attn_out^T
            boutbc = p_c.tile([P, D], F32)
            nc.sync.dma_start(boutbc[:], bout_ap[None, :].to_broadcast((P, D)))

            epool = cs.enter_context(tc.tile_pool(name="exp", bufs=exp_bufs))
            psS = cs.enter_context(
                tc.tile_pool(name="psS", bufs=psS_bufs, space="PSUM")
            )
            psO = cs.enter_context(
                tc.tile_pool(name="psO", bufs=psO_bufs, space="PSUM")
            )
            psY = cs.enter_context(
                tc.tile_pool(name="psY", bufs=psY_bufs, space="PSUM")
            )
            rpool = cs.enter_context(tc.tile_pool(name="rp", bufs=2))
            bcpool = cs.enter_context(tc.tile_pool(name="bc", bufs=2))
            ypool = cs.enter_context(tc.tile_pool(name="yp", bufs=3))

            def scores_steps(j, pair):
                """Generator: emit one (scores -> exp -> mask) step per next()
                call, yielding the (e tile, dead) the attnV stage needs."""
                nkc = (j + 1) * NQ // P
                m = pair
                halves = [(0, 2 * pair), (64, 2 * pair + 1)]
                for i in range(nkc):
                    i_loc = i - 4 * j
                    dead = max(0, i_loc * P)  # causally-dead columns
                    ps = psS.tile(
                        [P, 2 * NQ], F32, tag="psS", name=f"psS_{j}_{m}_{i}"
                    )
                    ps_v = ps[:].rearrange("p (g c) -> p g c", c=NQ)
                    for idx, (base, h) in enumerate(halves):
                        nc.tensor.matmul(
                            ps_v[:, idx, dead:],
                            qkT[base : base + 64, 8 + m, i * P : (i + 1) * P],
                            qkT[
                                base : base + 64,
                                m,
                                j * NQ + dead : (j + 1) * NQ,
                            ],
                            start=True,
                            stop=True,
                        )
                    e = epool.tile(
                        [P, 2 * NQ], BF16, tag="exp", name=f"e_{j}_{m}_{i}"
                    )
                    e_v = e[:].rearrange("p (g c) -> p g c", c=NQ)
                    nc.scalar.activation(
                        e_v[:, :, dead:],
                        ps_v[:, :, dead:],
                        mybir.ActivationFunctionType.Exp,
                        scale=0.125,
                    )
                    if i_loc >= 0:
                        # mask the 128-wide diagonal band: keep e[p, c]
                        # only where c >= p
                        nc.vector.tensor_tensor(
                            out=e_v[:, :, dead : dead + P],
                            in0=e_v[:, :, dead : dead + P],
                            in1=tri2[:].rearrange("p (g c) -> p g c", c=P),
                            op=mybir.AluOpType.mult,
                        )
                    yield (e, dead)

            def emit_attnv_drain(j, pair, es):
                m = pair
                halves = [(0, 2 * pair), (64, 2 * pair + 1)]
                nkc = len(es)
                po = {}
                for base, h in halves:
                    po[h] = psO.tile([65, NQ], F32, tag="psO", name=f"psO_{j}_{h}")
                for i, (e, dead) in enumerate(es):
                    for idx, (base, h) in enumerate(halves):
                        # columns < dead get no contribution from this k
                        # chunk (causally dead) -- skip them in the matmul
                        nc.tensor.matmul(
                            po[h][:, dead:],
                            v_sb[:, i, 65 * h : 65 * h + 65],
                            e[:, idx * NQ + dead : (idx + 1) * NQ],
                            start=(i == 0),
                            stop=(i == nkc - 1),
                        )
                for base, h in halves:
                    src_o = po[h]
                    rt = rpool.tile([1, NQ], F32, tag="r", name=f"r_{j}_{h}")
                    nc.vector.reciprocal(rt[:], src_o[64:65, :])
                    rbc = bcpool.tile([64, NQ], F32, tag="rbc", name=f"rbc_{j}_{h}")
                    nc.gpsimd.partition_broadcast(rbc[:], rt[:], channels=64)
                    # normalize on DVE (gpsimd cannot read PSUM)
                    nc.vector.tensor_tensor(
                        out=attnT[base : base + 64, m, j * NQ : (j + 1) * NQ],
                        in0=src_o[0:64, :],
                        in1=rbc[:],
                        op=mybir.AluOpType.mult,
                    )

            def emit_d_chunk(qc, nq, pool=None, tag="psY"):
                pool = pool or psY
                ps = pool.tile([P, NQ], F32, tag=tag, name=f"psYc_{nq}_{qc}")
                for kc in range(DC):
                    nc.tensor.matmul(
                        ps[:],
                        attnT[:, kc, qc * P : (qc + 1) * P],
                        wo_half[nq][:, kc, :],
                        start=(kc == 0),
                        stop=(kc == DC - 1),
                    )
                yt = ypool.tile([P, NQ], F32, tag="y", name=f"y_{nq}_{qc}")
                nc.vector.tensor_tensor(
                    out=yt[:],
                    in0=ps[:],
                    in1=boutbc[:, nq * NQ : (nq + 1) * NQ],
                    op=mybir.AluOpType.add,
                )
                nc.sync.dma_start(
                    y_ap[qc * P : (qc + 1) * P, nq * NQ : (nq + 1) * NQ], yt[:]
                )

            NP = H // 2
            for j in range(S // NQ):
                # software-pipelined pairs: prime the next pair's first two
                # score chunks, run attnV+drain of the current pair (and a
                # phase-D chunk of the previous window), then finish the next
                # pair's scores while ACT works ahead.
                gen = scores_steps(j, 0)
                es_cur = list(gen)
                for p in range(NP):
                    es_next = None
                    if p + 1 < NP:
                        gen = scores_steps(j, p + 1)
                        es_next = [next(gen), next(gen)]
                    emit_attnv_drain(j, p, es_cur)
                    if j == 0 and p % 2 == 1:
                        # window 0 only reads V of keys 0-511: compute the
                        # other half of V here, under window 0's ACT shadow
                        emit_b2_so(4 + p // 2, psY, "psY")
                    if interleave_d and j == 1:
                        # spread window-0's D chunks across window 1's pairs
                        emit_d_chunk(p // 2, p % 2)
                    if es_next is not None:
                        es_next.extend(gen)
                    es_cur = es_next

            # last window's D has nothing left to hide behind -- borrow the
            # now-idle psS slots for extra pipelining
            tail_windows = [1] if interleave_d else [0, 1]
            for jw in tail_windows:
                for qc in range(4 * jw, 4 * jw + 4):
                    for nq in range(D // NQ):
                        k = qc * 2 + nq
                        if k % 3 == 0:
                            emit_d_chunk(qc, nq)
                        else:
                            emit_d_chunk(qc, nq, pool=psS, tag="psS")

        wvp.release()
        p_ab.release()
        wobp.release()
        top.close()

    nc.compile()
    return nc


_CACHED = {}


def _get_nc():
    if "nc" not in _CACHED:
        _CACHED["nc"] = build_kernel(use_f32r=True, niter=1)
    return _CACHED["nc"]


def kernel(x, W_qkv, b_qkv, W_out, b_out):
    x = np.ascontiguousarray(np.asarray(x, dtype=np.float32))
    W_qkv = np.ascontiguousarray(np.asarray(W_qkv, dtype=np.float32))
    b_qkv = np.ascontiguousarray(np.asarray(b_qkv, dtype=np.float32))
    W_out = np.ascontiguousarray(np.asarray(W_out, dtype=np.float32))
    b_out = np.ascontiguousarray(np.asarray(b_out, dtype=np.float32))
    B = x.shape[0]
    assert x.shape == (8, S, D), f"expected x [8, {S}, {D}], got {x.shape}"

    from concourse.bass_utils import run_bass_kernel_spmd

    nc = _get_nc()
    in_maps = [
        {
            "x": np.ascontiguousarray(x[b]),
            "W_qkv": W_qkv,
            "b_qkv": b_qkv,
            "W_out": W_out,
            "b_out": b_out,
        }
        for b in range(B)
    ]
    res = run_bass_kernel_spmd(nc, in_maps, list(range(B)))
    return np.stack([res.results[b]["y"] for b in range(B)]).astype(np.float32)
